# revision 4
# baseline (speedup 1.0000x reference)
"""Trainium2 Bass kernel for the stacked-LSTM model (nn_Model2_16904991277618).

Model: LSTM-A(64->40, return_sequences) -> LSTM-B(40->40, last) over T=1024,
plus a small dense tail on `feat`, concat, 3 dense layers -> sigmoid [B,1].

Strategy (v2): data-parallel over batch (B=512 -> 64 rows/core), feature-major
layout, and a macro-step that fuses cell A at step t with cell B at step t-2.

Per macro-step engine budget:
  PE : 4 matmuls, one per (cell, gate-pair). K packs recurrent h + bias
       ones-row + input x in one rhs, so each gate-pair is a single matmul.
  Act: 2 ops. One big Sigmoid over all gates of both cells ([128, 4*BC]);
       the g-gate is pre-scaled by 2 in the weights so tanh(g) = 2*sig(2g)-1
       comes out of the same op. One Tanh(scale=2) over the half-cell-state.
  DVE: 5 bf16 ops: m2 = (sg-.5)*si (fused scalar_tensor_tensor),
       p = sf*c_half, c_half' = m2 + p, hA = soA*tcA, hB = soB*tcB.
  DMA: 1 small copy per step moving hA into the B-cell rhs ring (6-step
       slack hides DMA latency).

Gate row bands (partition-base rules: SBUF-SBUF operand pairs must share
base; out base may differ): f,o at rows 0:40; i,g at rows 64:104.
rhs layout rows: 0:40 = h(recurrent), 40 = ones (bias), 64:128 = x input.
h is written by the DVE directly into the staged x chunk for the next step,
so the A-cell recurrence needs no copies at all.
"""

import functools
import os
import sys

import numpy as np

for _p in ("/opt/trn_rl_repo", "/root/.axon_site/_ro/trn_rl_repo"):
    if os.path.isdir(_p) and _p not in sys.path:
        sys.path.insert(0, _p)

import ml_dtypes  # noqa: E402

import concourse.bass as bass  # noqa: E402
import concourse.bacc as bacc  # noqa: E402
import concourse.mybir as mybir  # noqa: E402
import concourse.tile as tile  # noqa: E402
from concourse.bass_utils import run_bass_kernel_spmd  # noqa: E402

F32 = mybir.dt.float32
BF16 = mybir.dt.bfloat16
AF = mybir.ActivationFunctionType
OP = mybir.AluOpType

NCORES = 8
H = 40
D = 10
F = 64

# gate column ranges in the reference [*, 4H] weight matrices
_I, _Fg, _G, _O = slice(0, 40), slice(40, 80), slice(80, 120), slice(120, 160)


def _bf(x):
    return np.ascontiguousarray(x, dtype=ml_dtypes.bfloat16)


def _f32c(x):
    return np.ascontiguousarray(x, dtype=np.float32)


def _wpack(Wk, Wr, b, s_top, s_bot, krows, scale_bot=1.0):
    """lhsT [krows, 128]: gate s_top at cols 0:40, s_bot at cols 64:104.

    Rows 0:40 = Wr (recurrent h), row 40 = bias, rows 64:64+kx = Wk (input x).
    Rows 41:64 (and any tail) are zero so garbage rhs rows contribute 0.
    """
    Wk = np.asarray(Wk, np.float32)
    Wr = np.asarray(Wr, np.float32)
    b = np.asarray(b, np.float32)
    kx = Wk.shape[0]
    out = np.zeros((krows, 128), np.float32)
    for cols, sl, sc in (((0, 40), s_top, 1.0), ((64, 104), s_bot, scale_bot)):
        out[0:40, cols[0]:cols[1]] = sc * Wr[:, sl]
        out[40, cols[0]:cols[1]] = sc * b[sl]
        out[64:64 + kx, cols[0]:cols[1]] = sc * Wk[:, sl]
    return _bf(out)


def _build_program(T, BC):
    CHUNK_T = min(T, 128)
    n_chunks = T // CHUNK_T
    assert n_chunks * CHUNK_T == T
    BC2, BC3, BC4 = 2 * BC, 3 * BC, 4 * BC
    NRING = 8

    nc = bacc.Bacc("TRN2", debug=False, target_bir_lowering=False,
                   num_devices=NCORES)

    def din(name, shape, dt):
        return nc.dram_tensor(name, list(shape), dt, kind="ExternalInput").ap()

    xt = din("xt", (F, T * BC), BF16)
    featT = din("featT", (F, BC), BF16)
    d_in = {
        "wa_fi": din("wa_fi", (128, 128), BF16),
        "wa_og": din("wa_og", (128, 128), BF16),
        "wb_fi": din("wb_fi", (104, 128), BF16),
        "wb_og": din("wb_og", (104, 128), BF16),
        "wg": din("wg", (F, D), BF16),
        "wh": din("wh", (D, D), BF16),
        "wc": din("wc", (74, 2 * D), BF16),
        "wd": din("wd", (2 * D, D), BF16),
        "wo": din("wo", (D, 1), BF16),
        "bg": din("bg", (D, 1), F32),
        "bh": din("bh", (D, 1), F32),
        "bc2": din("bc2", (2 * D, 1), F32),
        "bd": din("bd", (D, 1), F32),
        "bo": din("bo", (1, 1), F32),
    }

    out_dram = nc.dram_tensor("out", [1, BC], F32, kind="ExternalOutput").ap()

    from contextlib import ExitStack

    with tile.TileContext(nc) as tc:
        with ExitStack() as ctx:
            wpool = ctx.enter_context(tc.tile_pool(name="w", bufs=1))
            xpool = ctx.enter_context(tc.tile_pool(name="x", bufs=1))
            gpool = ctx.enter_context(tc.tile_pool(name="g", bufs=3))
            cpool = ctx.enter_context(tc.tile_pool(name="c", bufs=2))
            tpool = ctx.enter_context(tc.tile_pool(name="t", bufs=3))
            spool = ctx.enter_context(tc.tile_pool(name="s", bufs=1))
            psum = ctx.enter_context(tc.tile_pool(name="ps", bufs=3,
                                                  space="PSUM"))

            W = {}
            for nm, src in d_in.items():
                t_ = wpool.tile(list(src.shape), src.dtype, name=f"w_{nm}")
                nc.sync.dma_start(t_[:], src[:])
                W[nm] = t_
            ftile = wpool.tile([F, BC], BF16, name="w_featT")
            nc.sync.dma_start(ftile[:], featT[:])

            # x chunks: rows 64:128 = x features (DMA), row 40 = ones,
            # rows 0:40 = hA(t-1) written on-chip per step.
            xch = []
            for ci in range(n_chunks):
                xc = xpool.tile([128, CHUNK_T * BC], BF16, name=f"xc{ci}",
                                tag=f"xc{ci}")
                nc.sync.dma_start(
                    xc[64:128, :],
                    xt[:, ci * CHUNK_T * BC:(ci + 1) * CHUNK_T * BC])
                # ones row lives at partition 40; engine APs need 32-aligned
                # partition bases, so set rows 32:64 to 1 (41:64 are unused,
                # 32:40 get overwritten by hA before any matmul reads them)
                nc.gpsimd.memset(xc[32:64, :], 1.0)
                xch.append(xc)
            # t=0 block: hA(-1) = 0 (rows 0:40), keep ones at row 40
            nc.gpsimd.memset(xch[0][0:32, 0:BC], 0.0)
            nc.gpsimd.memset(xch[0][32:40, 0:BC], 0.0)

            # B-cell rhs ring: rows 0:40 = hB, row 40 = ones, 64:104 = hA.
            ring = []
            for k in range(NRING):
                rb = xpool.tile([128, BC], BF16, name=f"rb{k}", tag=f"rb{k}")
                nc.gpsimd.memset(rb[0:32, :], 0.0)
                nc.gpsimd.memset(rb[32:64, :], 1.0)
                nc.gpsimd.memset(rb[32:40, :], 0.0)
                ring.append(rb)

            hsc = wpool.tile([128, BC], BF16, name="hsc")  # hA(T-1) scratch
            zcat = spool.tile([74, BC], BF16, name="zcat")
            nc.gpsimd.memset(zcat[:], 0.0)

            c_prev = cpool.tile([40, BC2], BF16, name="c_init", tag="c")
            nc.gpsimd.memset(c_prev[:], 0.0)

            for s in range(T + 2):
                t, tau = s, s - 2
                A = t < T
                Bact = tau >= 0

                zz = psum.tile([128, BC4], F32, name=f"zz{s}", tag="zz")
                if A:
                    ci, tl = divmod(t, CHUNK_T)
                    rhsA = xch[ci][0:128, tl * BC:(tl + 1) * BC]
                    nc.tensor.matmul(zz[:, 0:BC], W["wa_fi"][:], rhsA,
                                     start=True, stop=True)
                    nc.tensor.matmul(zz[:, BC2:BC3], W["wa_og"][:], rhsA,
                                     start=True, stop=True)
                if Bact:
                    rhsB = ring[tau % NRING][0:104, :]
                    nc.tensor.matmul(zz[:, BC:BC2], W["wb_fi"][:], rhsB,
                                     start=True, stop=True)
                    nc.tensor.matmul(zz[:, BC3:BC4], W["wb_og"][:], rhsB,
                                     start=True, stop=True)

                gp = gpool.tile([128, BC4], BF16, name=f"gp{s}", tag="gp")
                nc.scalar.activation(gp[:], zz[:], AF.Sigmoid)

                m2 = tpool.tile([40, BC2], BF16, name=f"m2_{s}", tag="m2")
                p = tpool.tile([40, BC2], BF16, name=f"p{s}", tag="p")
                cn = cpool.tile([40, BC2], BF16, name=f"c{s}", tag="c")
                tct = tpool.tile([40, BC2], BF16, name=f"tc{s}", tag="tc")

                # hA destination: staged x of step t+1, or scratch at t=T-1
                if A:
                    if t + 1 < T:
                        ci2, tl2 = divmod(t + 1, CHUNK_T)
                        hA_dst = xch[ci2][0:40, tl2 * BC:(tl2 + 1) * BC]
                    else:
                        hA_dst = hsc[0:40, :]

                if A and Bact:
                    nc.vector.scalar_tensor_tensor(
                        m2[:], gp[64:104, BC2:BC4], -0.5,
                        gp[64:104, 0:BC2], OP.add, OP.mult)
                    nc.vector.tensor_tensor(p[:], gp[0:40, 0:BC2],
                                            c_prev[:], OP.mult)
                    nc.vector.tensor_tensor(cn[:], m2[:], p[:], OP.add)
                    nc.scalar.activation(tct[:], cn[:], AF.Tanh, scale=2.0)
                    nc.vector.tensor_tensor(hA_dst, gp[0:40, BC2:BC3],
                                            tct[:, 0:BC], OP.mult)
                    hB_dst = ring[(tau + 1) % NRING][0:40, :]
                    nc.vector.tensor_tensor(hB_dst, gp[0:40, BC3:BC4],
                                            tct[:, BC:BC2], OP.mult)
                elif A:  # lead-in: cell A only
                    nc.vector.scalar_tensor_tensor(
                        m2[:, 0:BC], gp[64:104, BC2:BC3], -0.5,
                        gp[64:104, 0:BC], OP.add, OP.mult)
                    nc.vector.tensor_tensor(p[:, 0:BC], gp[0:40, 0:BC],
                                            c_prev[:, 0:BC], OP.mult)
                    nc.vector.tensor_tensor(cn[:, 0:BC], m2[:, 0:BC],
                                            p[:, 0:BC], OP.add)
                    nc.gpsimd.memset(cn[:, BC:BC2], 0.0)
                    nc.scalar.activation(tct[:, 0:BC], cn[:, 0:BC], AF.Tanh,
                                         scale=2.0)
                    nc.vector.tensor_tensor(hA_dst, gp[0:40, BC2:BC3],
                                            tct[:, 0:BC], OP.mult)
                else:  # lead-out: cell B only
                    nc.vector.scalar_tensor_tensor(
                        m2[:, BC:BC2], gp[64:104, BC3:BC4], -0.5,
                        gp[64:104, BC:BC2], OP.add, OP.mult)
                    nc.vector.tensor_tensor(p[:, BC:BC2], gp[0:40, BC:BC2],
                                            c_prev[:, BC:BC2], OP.mult)
                    nc.vector.tensor_tensor(cn[:, BC:BC2], m2[:, BC:BC2],
                                            p[:, BC:BC2], OP.add)
                    nc.scalar.activation(tct[:, BC:BC2], cn[:, BC:BC2],
                                         AF.Tanh, scale=2.0)
                    hB_dst = (zcat[0:40, :] if s == T + 1
                              else ring[(tau + 1) % NRING][0:40, :])
                    nc.vector.tensor_tensor(hB_dst, gp[0:40, BC3:BC4],
                                            tct[:, BC:BC2], OP.mult)

                if A:  # hA(t) -> ring slot for B-step t (read at s=t+2)
                    nc.sync.dma_start(ring[t % NRING][64:104, :], hA_dst)

                c_prev = cn

            # ---- dense tail ----
            # zcat [74, BC]: hB at rows 0:40 (written by last hB op),
            # y at rows 64:74 (wc re-packed to match)
            ps1 = psum.tile([D, BC], F32, name="ps1", tag="zz")
            nc.tensor.matmul(ps1[:], W["wg"][:], ftile[:],
                             start=True, stop=True)
            y1 = spool.tile([D, BC], BF16, name="y1")
            nc.scalar.activation(y1[:], ps1[:], AF.Tanh, bias=W["bg"][:])

            ps2 = psum.tile([D, BC], F32, name="ps2", tag="zz")
            nc.tensor.matmul(ps2[:], W["wh"][:], y1[:], start=True, stop=True)
            nc.scalar.activation(zcat[64:74, :], ps2[:], AF.Tanh,
                                 bias=W["bh"][:])

            ps3 = psum.tile([2 * D, BC], F32, name="ps3", tag="zz")
            nc.tensor.matmul(ps3[:], W["wc"][:], zcat[:], start=True,
                             stop=True)
            c1 = spool.tile([2 * D, BC], BF16, name="c1")
            nc.scalar.activation(c1[:], ps3[:], AF.Relu, bias=W["bc2"][:])

            ps4 = psum.tile([D, BC], F32, name="ps4", tag="zz")
            nc.tensor.matmul(ps4[:], W["wd"][:], c1[:], start=True, stop=True)
            d1 = spool.tile([D, BC], BF16, name="d1")
            nc.scalar.activation(d1[:], ps4[:], AF.Relu, bias=W["bd"][:])

            ps5 = psum.tile([1, BC], F32, name="ps5", tag="zz")
            nc.tensor.matmul(ps5[:], W["wo"][:], d1[:], start=True, stop=True)
            osb = spool.tile([1, BC], F32, name="osb")
            nc.scalar.activation(osb[:], ps5[:], AF.Sigmoid, bias=W["bo"][:])

            nc.sync.dma_start(out_dram[:], osb[:])

    nc.compile()
    return nc


@functools.lru_cache(maxsize=2)
def _program(T, BC):
    return _build_program(T, BC)


def _prep_shared(Wa_k, Wa_r, ba, Wb_k, Wb_r, bb, Wg, bg, Wh, bh, Wc, bc, Wd,
                 bd, Wo, bo):
    wc_re = np.zeros((74, 2 * D), np.float32)
    wc_re[0:40] = np.asarray(Wc, np.float32)[0:40]
    wc_re[64:74] = np.asarray(Wc, np.float32)[40:50]
    return {
        "wa_fi": _wpack(Wa_k, Wa_r, ba, _Fg, _I, 128),
        "wa_og": _wpack(Wa_k, Wa_r, ba, _O, _G, 128, scale_bot=2.0),
        "wb_fi": _wpack(Wb_k, Wb_r, bb, _Fg, _I, 104),
        "wb_og": _wpack(Wb_k, Wb_r, bb, _O, _G, 104, scale_bot=2.0),
        "wg": _bf(Wg), "wh": _bf(Wh), "wc": _bf(wc_re), "wd": _bf(Wd),
        "wo": _bf(Wo),
        "bg": _f32c(np.asarray(bg)[:, None]),
        "bh": _f32c(np.asarray(bh)[:, None]),
        "bc2": _f32c(np.asarray(bc)[:, None]),
        "bd": _f32c(np.asarray(bd)[:, None]),
        "bo": _f32c(np.asarray(bo)[:, None]),
    }


def _prep_seq(seq, T, BC):
    # [core, F, T*BC]: row f, col t*BC + b
    arr = np.asarray(seq, np.float32).reshape(NCORES, BC, T, F)
    arr = arr.transpose(0, 3, 2, 1).reshape(NCORES, F, T * BC)
    return _bf(arr)


def kernel(seq, feat, Wa_k, Wa_r, ba, Wb_k, Wb_r, bb, Wg, bg, Wh, bh, Wc, bc,
           Wd, bd, Wo, bo, _trace=False):
    seq = np.asarray(seq)
    feat = np.asarray(feat)
    B, T, _ = seq.shape
    assert B % NCORES == 0
    BC = B // NCORES

    nc = _program(T, BC)

    shared = _prep_shared(Wa_k, Wa_r, ba, Wb_k, Wb_r, bb, Wg, bg, Wh, bh, Wc,
                          bc, Wd, bd, Wo, bo)
    xt = _prep_seq(seq, T, BC)
    featc = np.asarray(feat, np.float32).reshape(NCORES, BC, F)

    in_maps = []
    for c in range(NCORES):
        m = dict(shared)
        m["xt"] = xt[c]
        m["featT"] = _bf(featc[c].T)
        in_maps.append(m)

    res = run_bass_kernel_spmd(nc, in_maps, core_ids=list(range(NCORES)),
                               trace=_trace)
    out = np.concatenate([res.results[c]["out"][0] for c in range(NCORES)])
    out = out.astype(np.float32).reshape(B, 1)
    if _trace:
        kernel.last_results = res
    return out
